# revision 6
# baseline (speedup 1.0000x reference)
"""GAT conv layer (B=2, N=4096, C=256, H=4, D=64) on 8 TRN2 NeuronCores.

Sharding: core c handles batch b = c//4 and target-node shard s = c%4
(1024 target nodes), all 4 heads.  Per core, scores live in
[j = source node (partition), i = target node (free)] layout so that
softmax normalization needs no on-chip reduction at all: the attention
matmul uses an augmented stationary operand [Wh | 1] whose ones column
accumulates the softmax denominator alongside the numerator.

Score pipeline per [128, 4096] fp16 bigtile (DVE scalar_tensor_tensor):
  z = (srcB + tgt[j]) - m255          one STT  (e-build + mask fuse)
  l = max(0.2*z, z)                   one STT  (= leaky_relu; masked rows
                                      land at 0.2*e - 51 -> exp ~ 1e-22)
  p = Exp(l)                          one ACT pass -> f32
  psum[65,1024] += [Wh|1].T @ p       PE, fp32
Then per 128-target block: PE-transpose, reciprocal of the denominator
row, scale + accumulate heads (0.25 head-mean factor is baked into the
projection weights on the host).
"""

import numpy as np

B, N, C, H, D = 2, 4096, 256, 4, 64
NEG = 0.2
SH = N // 4          # 1024 target nodes per core
JC = N // 128        # 32 source chunks
G = JC // 4          # 8 chunk-groups (bigtiles of [128, 4096])
MASKV = 255.0

_cached = {}


def _build(reps=1):
    import concourse.bacc as bacc
    import concourse.tile as tile
    from concourse import mybir
    from concourse.masks import make_identity

    f32 = mybir.dt.float32
    f16 = mybir.dt.float16
    Alu = mybir.AluOpType

    nc = bacc.Bacc(None, target_bir_lowering=False, name="gat")

    xT = nc.dram_tensor("xT", [2, 128, N], f32, kind="ExternalInput")
    xTs = nc.dram_tensor("xTs", [2, 128, SH], f32, kind="ExternalInput")
    waug = nc.dram_tensor("waug", [2, 128, H * 65], f32, kind="ExternalInput")
    wsb = nc.dram_tensor("wsb", [H, 2, 128, 128], f32, kind="ExternalInput")
    m255 = nc.dram_tensor("m255", [N, SH], f16, kind="ExternalInput")
    outd = nc.dram_tensor("out", [SH, D], f32, kind="ExternalOutput")

    def pipeline(tc, const, whaug, tgt_all, tgt16, srcB, ident):
        # ---------------- phase A: projection  Wh|tgt = x @ waug ----------
        with tc.tile_pool(name="ld", bufs=1) as ld, \
             tc.tile_pool(name="psA", bufs=4, space="PSUM") as psA:
            xT_sb = ld.tile([128, 2 * N], f32)
            waug_sb = ld.tile([128, 2 * H * 65], f32)
            xTs_sb = ld.tile([128, 2 * SH], f32)
            wsb_sb = ld.tile([128, H * 2 * 128], f32)
            for cc in range(2):
                nc.sync.dma_start(xT_sb[:, cc * N:(cc + 1) * N], xT[cc])
                nc.sync.dma_start(
                    waug_sb[:, cc * H * 65:(cc + 1) * H * 65], waug[cc])
                nc.sync.dma_start(xTs_sb[:, cc * SH:(cc + 1) * SH], xTs[cc])
                for h in range(H):
                    nc.sync.dma_start(
                        wsb_sb[:, (h * 2 + cc) * 128:(h * 2 + cc + 1) * 128],
                        wsb[h, cc])

            for jc in range(JC):
                psp = psA.tile([128, H * 65], f32, tag="psp")
                for cc in range(2):
                    nc.tensor.matmul(
                        psp,
                        xT_sb[:, cc * N + jc * 128: cc * N + (jc + 1) * 128],
                        waug_sb[:, cc * H * 65:(cc + 1) * H * 65],
                        start=(cc == 0), stop=(cc == 1))
                for h in range(H):
                    nc.scalar.copy(
                        whaug[:, (jc * H + h) * 65:(jc * H + h) * 65 + 64],
                        psp[:, h * 65: h * 65 + 64])
                # tgt columns live at h*65+64; strided gather of all 4
                nc.scalar.copy(tgt_all[:, jc * H:(jc + 1) * H], psp[:, 64::65])
            nc.scalar.copy(tgt16, tgt_all)

            # -------------- phase B: srcB = broadcast(x_shard @ wsrc) -----
            with tc.tile_pool(name="psB", bufs=2, space="PSUM") as psB:
                for h in range(H):
                    pss = psB.tile([128, SH], f32, tag="pss")
                    for half in range(2):
                        for cc in range(2):
                            nc.tensor.matmul(
                                pss[:, half * 512:(half + 1) * 512],
                                wsb_sb[:, (h * 2 + cc) * 128:(h * 2 + cc + 1) * 128],
                                xTs_sb[:, cc * SH + half * 512: cc * SH + (half + 1) * 512],
                                start=(cc == 0), stop=(cc == 1))
                    nc.scalar.copy(srcB[:, h * SH:(h + 1) * SH], pss)

        # ---------------- phase C: scores + attention matmul --------------
        with tc.tile_pool(name="mw", bufs=2) as mw, \
             tc.tile_pool(name="zw", bufs=2) as zw, \
             tc.tile_pool(name="lw", bufs=2) as lw, \
             tc.tile_pool(name="pw", bufs=2) as pw, \
             tc.tile_pool(name="psC", bufs=1, space="PSUM") as psC, \
             tc.tile_pool(name="nd", bufs=1) as ndp:
            acc = [psC.tile([65, SH], f32, name=f"acc{h}", tag=f"acc{h}")
                   for h in range(H)]
            for g in range(G):
                m_t = mw.tile([128, 4096], f16, tag="m")
                nc.sync.dma_start(
                    m_t.rearrange("p (c i) -> p c i", c=4),
                    m255[g * 512:(g + 1) * 512, :].rearrange(
                        "(c p) i -> p c i", p=128))
                for h in range(H):
                    z_t = zw.tile([128, 4096], f16, tag="z")
                    for jl in range(4):
                        jc = g * 4 + jl
                        nc.vector.scalar_tensor_tensor(
                            out=z_t[:, jl * SH:(jl + 1) * SH],
                            in0=srcB[:, h * SH:(h + 1) * SH],
                            scalar=tgt16[:, jc * H + h: jc * H + h + 1],
                            in1=m_t[:, jl * SH:(jl + 1) * SH],
                            op0=Alu.add, op1=Alu.subtract)
                    l_t = lw.tile([128, 4096], f16, tag="l")
                    nc.vector.scalar_tensor_tensor(
                        out=l_t, in0=z_t, scalar=NEG, in1=z_t,
                        op0=Alu.mult, op1=Alu.max)
                    p_t = pw.tile([128, 4096], f32, tag="p")
                    nc.scalar.activation(
                        out=p_t, in_=l_t,
                        func=mybir.ActivationFunctionType.Exp)
                    for jl in range(4):
                        jc = g * 4 + jl
                        for half in range(2):
                            nc.tensor.matmul(
                                acc[h][:, half * 512:(half + 1) * 512],
                                whaug[:, (jc * H + h) * 65:(jc * H + h + 1) * 65],
                                p_t[:, jl * SH + half * 512: jl * SH + (half + 1) * 512],
                                start=(jc == 0), stop=(jc == JC - 1),
                                skip_group_check=True)

            # ---------------- evacuate accumulators ----------------------
            nd = [ndp.tile([65, SH], f32, name=f"nd{h}", tag=f"nd{h}")
                  for h in range(H)]
            for h in range(H):
                nc.scalar.copy(nd[h], acc[h])

        # ---------------- phase D: normalize + head mean ------------------
        with tc.tile_pool(name="psD", bufs=4, space="PSUM") as psD, \
             tc.tile_pool(name="oc", bufs=3) as oc, \
             tc.tile_pool(name="rc", bufs=4) as rc:
            for blk in range(SH // 128):
                o_prev = None
                for h in range(H):
                    trp = psD.tile([128, 65], f32, tag="trp")
                    nc.tensor.transpose(
                        trp, nd[h][:, blk * 128:(blk + 1) * 128], ident)
                    rec = rc.tile([128, 1], f32, tag="rec")
                    nc.vector.reciprocal(rec, trp[:, 64:65])
                    o_t = oc.tile([128, D], f32, tag="o")
                    if o_prev is None:
                        nc.vector.tensor_scalar_mul(
                            out=o_t, in0=trp[:, 0:D], scalar1=rec)
                    else:
                        nc.vector.scalar_tensor_tensor(
                            out=o_t, in0=trp[:, 0:D], scalar=rec,
                            in1=o_prev, op0=Alu.mult, op1=Alu.add)
                    o_prev = o_t
                nc.sync.dma_start(outd[blk * 128:(blk + 1) * 128, :], o_prev)

    with tile.TileContext(nc) as tc:
        with tc.tile_pool(name="const", bufs=1) as const:
            whaug = const.tile([128, JC * H * 65], f32)
            nc.vector.memset(whaug, 1.0)
            tgt_all = const.tile([128, JC * H], f32)
            tgt16 = const.tile([128, JC * H], f16)
            srcB = const.tile([128, H * SH], f16)
            ident = const.tile([65, 65], f32)
            make_identity(nc, ident)
            # reps>1 replicates the pipeline for tunnel-free HW timing:
            # t_hw = (wall(reps=R) - wall(reps=1)) / (R - 1)
            for _rep in range(reps):
                pipeline(tc, const, whaug, tgt_all, tgt16, srcB, ident)

    nc.compile()
    return nc


def _prep_inputs(x, adj_matrix_masked, W, attention):
    """Host-side shard/layout prep (slicing, transposes, weight packing)."""
    x = np.ascontiguousarray(x, dtype=np.float32)
    W = np.ascontiguousarray(W, dtype=np.float32)
    attention = np.ascontiguousarray(attention, dtype=np.float32)

    a_src = attention[:, :D, 0]          # [H, D]
    a_tgt = attention[:, D:, 0]          # [H, D]
    Wh_cols = W.reshape(C, H, D)
    w_src = np.einsum("chd,hd->ch", Wh_cols, a_src)   # [C, H]
    w_tgt = np.einsum("chd,hd->ch", Wh_cols, a_tgt)   # [C, H]

    waug = np.zeros((C, H * 65), np.float32)
    for h in range(H):
        waug[:, h * 65: h * 65 + 64] = 0.25 * Wh_cols[:, h, :]
        waug[:, h * 65 + 64] = w_tgt[:, h]
    waug = np.ascontiguousarray(waug.reshape(2, 128, H * 65))

    wsb = np.empty((H, 2, 128, 128), np.float32)
    for h in range(H):
        wsb[h] = np.repeat(w_src[:, h][:, None], 128, axis=1).reshape(2, 128, 128)

    xT = np.empty((B, 2, 128, N), np.float32)
    for b in range(B):
        xT[b] = np.ascontiguousarray(x[b].T).reshape(2, 128, N)

    in_maps = []
    for c in range(8):
        b, s = c // 4, c % 4
        i0 = s * SH
        m = np.where(adj_matrix_masked[b, 0, i0:i0 + SH, :].T, np.float16(MASKV),
                     np.float16(0.0)).astype(np.float16)
        in_maps.append(dict(
            xT=xT[b],
            xTs=np.ascontiguousarray(xT[b][:, :, i0:i0 + SH]),
            waug=waug,
            wsb=wsb,
            m255=np.ascontiguousarray(m),
        ))
    return in_maps


def _run(x, adj_matrix_masked, W, attention, reps=1):
    from concourse.bass_utils import run_bass_kernel_spmd

    key = f"nc{reps}"
    if key not in _cached:
        _cached[key] = _build(reps)
    nc = _cached[key]

    in_maps = _prep_inputs(x, adj_matrix_masked, W, attention)
    res = run_bass_kernel_spmd(nc, in_maps, core_ids=list(range(8)))
    out = np.empty((B, N, D), np.float32)
    for c in range(8):
        b, s = c // 4, c % 4
        out[b, s * SH:(s + 1) * SH, :] = res.results[c]["out"]
    return out, res


def kernel(x, adj_matrix_masked, W, attention):
    out, _ = _run(x, adj_matrix_masked, W, attention)
    return out
